# revision 10
# baseline (speedup 1.0000x reference)
"""Trainium2 Bass kernel for nn_Classifier_55783035241052 (ROI pooling classifier).

Pipeline (reference):
  4 ROI crops of base_x[1,2048,128,128] -> adaptive_avg_pool 7x7 -> 1x1 conv
  2048->512 -> flatten [4,25088] -> d1 [4096,25088] relu -> d2 [2048,4096] relu
  -> d3 [21,2048] softmax / d4 [80,2048].

Sharding over 8 NeuronCores:
  - pooling/conv: channel-parallel (256 ch/core), pooling done as matmul with a
    host-built 0/1 binning matrix; conv contracts the local channels ->
    partial red [512,196]; AllReduce #1 sums partials.
  - d1: row-parallel (512 of 4096 outputs per core), weights streamed from HBM
    in a k'-permuted layout matching the on-chip red layout.
  - d2: contraction-parallel (512-col slice per core) producing partial
    h2^T [2048,4]; AllReduce #2 sums partials.
  - d3/d4 + softmax: replicated (tiny); host takes core 0's output.

kernel(**inputs) -> (out_class [4,21] f32, out_regr [4,80] f32)
"""
import numpy as np
import ml_dtypes

from concourse import bacc, mybir, tile
from concourse.bass_utils import run_bass_kernel_spmd

N_CORES = 8
POOL = 7
NBINS = POOL * POOL          # 49
NUM_ROIS = 4
FEAT = 2048
RED = 512
FLAT = RED * NBINS           # 25088
H1 = 2 * FEAT                # 4096
NCLS = 21
NREG = 4 * (NCLS - 1)        # 80
NOUT = NCLS + NREG           # 101

C_LOC = FEAT // N_CORES      # 256 channels per core
CB = C_LOC // 128            # 2 channel blocks per core
OB = RED // 128              # 4 conv-output blocks
J_LOC = H1 // N_CORES        # 512 d1 rows per core
JB = J_LOC // 128            # 4 j blocks
MB = FEAT // 128             # 16 h2 blocks
NCOLS = NUM_ROIS * NBINS     # 196 (r*49 + p) columns

F32 = mybir.dt.float32

_CACHE = {}


def _pool_meta(rois):
    """Per-ROI bin geometry. rois rows are (x, y, h, w); crop spatial dims
    are (S1, S2) = (w, h) in reference order."""
    metas = []
    for rid in range(NUM_ROIS):
        x, y, h, w = (int(v) for v in rois[rid])
        S1, S2 = w, h
        i = np.arange(POOL)
        hs = np.floor(i * S1 / POOL).astype(np.int64)
        he = np.ceil((i + 1) * S1 / POOL).astype(np.int64)
        ws = np.floor(i * S2 / POOL).astype(np.int64)
        we = np.ceil((i + 1) * S2 / POOL).astype(np.int64)
        px = S1 * S2
        pxp = ((px + 127) // 128) * 128
        B = np.zeros((pxp, NBINS), np.float32)
        u = np.arange(S1)
        v = np.arange(S2)
        row_in = (u[:, None] >= hs[None, :]) & (u[:, None] < he[None, :])  # [S1,7]
        col_in = (v[:, None] >= ws[None, :]) & (v[:, None] < we[None, :])  # [S2,7]
        # B[(u*S2+v), i*7+j] = row_in[u,i]*col_in[v,j]
        Bfull = (row_in[:, None, :, None] & col_in[None, :, None, :])
        B[:px] = Bfull.reshape(px, NBINS).astype(np.float32)
        area = ((he - hs)[:, None] * (we - ws)[None, :]).astype(np.float64)
        metas.append(dict(x=x, y=y, h=h, w=w, px=px, pxp=pxp, B=B,
                          inv_area=(1.0 / area).reshape(NBINS).astype(np.float32)))
    return metas


def _build(pxp_list, big_np, small_np, reps=1, with_cb=False, with_d1b=False,
           with_d2b=False, no_collectives=False):
    """Build the Bass program. pxp_list: padded pixel counts per ROI.
    big_np: dtype for bandwidth-heavy tensors (crops, B, d1w, redf);
    small_np: dtype for the remaining weights/activations."""
    BIG = mybir.dt.from_np(np.dtype(big_np))
    SML = mybir.dt.from_np(np.dtype(small_np))
    nc = bacc.Bacc("TRN2", target_bir_lowering=False, debug=False,
                   num_devices=N_CORES)

    # ---- DRAM parameters (per-core data arrives via in_maps) ----
    crop_d = [nc.dram_tensor(f"crop{r}", [128, (pxp_list[r] // 128) * C_LOC],
                             BIG, kind="ExternalInput") for r in range(NUM_ROIS)]
    b_d = [nc.dram_tensor(f"b{r}", [128, (pxp_list[r] // 128) * NBINS], BIG,
                          kind="ExternalInput") for r in range(NUM_ROIS)]
    invarea_d = nc.dram_tensor("inv_area", [128, NCOLS], F32, kind="ExternalInput")
    ident_d = nc.dram_tensor("ident", [128, 128], SML, kind="ExternalInput")
    convw_d = nc.dram_tensor("convw", [C_LOC, RED], F32, kind="ExternalInput")
    d1w_d = nc.dram_tensor("d1w", [128, (FLAT // 128) * J_LOC], BIG,
                           kind="ExternalInput")
    d2w_d = nc.dram_tensor("d2w", [128, JB * FEAT], SML, kind="ExternalInput")
    d34w_d = nc.dram_tensor("d34w", [128, MB * NOUT], SML, kind="ExternalInput")
    cb_d = nc.dram_tensor("cbias", [128, OB * NCOLS], F32,
                          kind="ExternalInput") if with_cb else None
    d1b_d = nc.dram_tensor("d1bias", [NUM_ROIS, J_LOC], F32,
                           kind="ExternalInput") if with_d1b else None
    d2b_d = nc.dram_tensor("d2bias", [128, MB * NUM_ROIS], F32,
                           kind="ExternalInput") if with_d2b else None
    out_d = nc.dram_tensor("out", [NUM_ROIS, NOUT], F32, kind="ExternalOutput")

    # Collective bounce buffers (internal DRAM; outputs must be Shared).
    ar1_in = [nc.dram_tensor(f"ar1_in{i}", [128, OB * NCOLS], F32)
              for i in range(reps)]
    ar1_out = [nc.dram_tensor(f"ar1_out{i}", [128, OB * NCOLS], F32,
                              addr_space="Shared") for i in range(reps)]
    ar2_in = [nc.dram_tensor(f"ar2_in{i}", [128, MB * NUM_ROIS], F32)
              for i in range(reps)]
    ar2_out = [nc.dram_tensor(f"ar2_out{i}", [128, MB * NUM_ROIS], F32,
                              addr_space="Shared") for i in range(reps)]

    is_bf16 = np.dtype(big_np).itemsize == 2
    CROP_G = 16   # px chunks per crop DMA
    D1_G = 8      # k-chunks per d1w DMA group (1 MB bf16)
    D1_BUFS = 8 if is_bf16 else 4
    CROP_BUFS = 3 if is_bf16 else 2
    n_k = FLAT // 128          # 196 contraction chunks for d1
    n_d1_groups = (n_k + D1_G - 1) // D1_G

    with tile.TileContext(nc) as tc:
        with (
            tc.tile_pool(name="const", bufs=1) as constp,
            tc.tile_pool(name="cropp", bufs=CROP_BUFS) as cropp,
            tc.tile_pool(name="bp", bufs=3) as bp,
            tc.tile_pool(name="d1wp", bufs=D1_BUFS) as d1wp,
            tc.tile_pool(name="work", bufs=2) as work,
            tc.tile_pool(name="psum", bufs=1, space="PSUM") as psum,
        ):
            # ---- resident constants ----
            invarea_sb = constp.tile([128, NCOLS], F32)
            nc.sync.dma_start(invarea_sb[:], invarea_d[:])
            ident_sb = constp.tile([128, 128], SML)
            nc.sync.dma_start(ident_sb[:], ident_d[:])
            convw_sb = [constp.tile([128, RED], F32, tag=f"convw{cb}", name=f"convw_sb{cb}")
                        for cb in range(CB)]
            for cb in range(CB):
                nc.sync.dma_start(convw_sb[cb][:],
                                  convw_d[cb * 128:(cb + 1) * 128, :])
            # (convw kept row-major: 2 KB contiguous rows)
            d2w_sb = [constp.tile([128, FEAT], SML, tag=f"d2w{jb}", name=f"d2w_sb{jb}")
                      for jb in range(JB)]
            for jb in range(JB):
                nc.sync.dma_start(d2w_sb[jb][:],
                                  d2w_d[:, jb * FEAT:(jb + 1) * FEAT])
            d34w_sb = constp.tile([128, MB * NOUT], SML)
            nc.sync.dma_start(d34w_sb[:], d34w_d[:])
            cb_sb = None
            if with_cb:
                cb_sb = constp.tile([128, OB * NCOLS], F32)
                nc.sync.dma_start(cb_sb[:], cb_d[:])
            d1b_sb = None
            if with_d1b:
                d1b_sb = constp.tile([NUM_ROIS, J_LOC], F32)
                nc.sync.dma_start(d1b_sb[:], d1b_d[:])
            d2b_sb = None
            if with_d2b:
                d2b_sb = constp.tile([128, MB * NUM_ROIS], F32)
                nc.sync.dma_start(d2b_sb[:], d2b_d[:])

            for it in range(reps):
                # ================= pooling (per ROI, per channel block) ====
                pooled_sb = [work.tile([128, NCOLS], F32, tag=f"pooled{cb}", name=f"pooled_sb{cb}")
                             for cb in range(CB)]
                for r in range(NUM_ROIS):
                    pxp = pxp_list[r]
                    nchunks = pxp // 128
                    pool_ps = [psum.tile([128, NBINS], F32, tag="accum", bufs=3,
                                         name=f"pool_ps{cb}")
                               for cb in range(CB)]
                    done = 0
                    while done < nchunks:
                        g = min(CROP_G, nchunks - done)
                        crop_t = cropp.tile([128, CROP_G * C_LOC], BIG, tag="crop")
                        nc.sync.dma_start(
                            crop_t[:, :g * C_LOC],
                            crop_d[r][:, done * C_LOC:(done + g) * C_LOC])
                        b_t = bp.tile([128, CROP_G * NBINS], BIG, tag="bmat")
                        nc.sync.dma_start(
                            b_t[:, :g * NBINS],
                            b_d[r][:, done * NBINS:(done + g) * NBINS])
                        for gi in range(g):
                            ci = done + gi
                            for cb in range(CB):
                                nc.tensor.matmul(
                                    pool_ps[cb][:],
                                    crop_t[:, gi * C_LOC + cb * 128:
                                           gi * C_LOC + (cb + 1) * 128],
                                    b_t[:, gi * NBINS:(gi + 1) * NBINS],
                                    start=(ci == 0), stop=(ci == nchunks - 1))
                        done += g
                    # scale by 1/area, cast to compute dtype
                    for cb in range(CB):
                        nc.vector.tensor_tensor(
                            pooled_sb[cb][:, r * NBINS:(r + 1) * NBINS],
                            pool_ps[cb][:],
                            invarea_sb[:, r * NBINS:(r + 1) * NBINS],
                            mybir.AluOpType.mult)

                # ================= 1x1 conv (local channels) ===============
                red_cat = work.tile([128, OB * NCOLS], F32, tag="red_cat")
                for ob in range(OB):
                    red_ps = psum.tile([128, NCOLS], F32, tag="accum", bufs=3)
                    for cb in range(CB):
                        nc.tensor.matmul(
                            red_ps[:],
                            convw_sb[cb][:, ob * 128:(ob + 1) * 128],
                            pooled_sb[cb][:],
                            start=(cb == 0), stop=(cb == CB - 1))
                    nc.vector.tensor_copy(
                        red_cat[:, ob * NCOLS:(ob + 1) * NCOLS], red_ps[:])

                # ================= AllReduce #1: red partials ==============
                nc.sync.dma_start(ar1_in[it][:], red_cat[:])
                if no_collectives:
                    nc.sync.dma_start(ar1_out[it][:], ar1_in[it][:])
                else:
                    nc.gpsimd.collective_compute(
                        "AllReduce", mybir.AluOpType.add,
                        ins=[ar1_in[it][:]], outs=[ar1_out[it][:]],
                        replica_groups=[list(range(N_CORES))])
                redsum = work.tile([128, OB * NCOLS], F32, tag="redsum")
                nc.sync.dma_start(redsum[:], ar1_out[it][:])
                if with_cb:
                    nc.vector.tensor_add(redsum[:], redsum[:], cb_sb[:])
                redf = work.tile([128, OB * NCOLS], BIG, tag="redf")
                nc.vector.tensor_copy(redf[:], redsum[:])

                # ================= d1: h1 = relu(flat' @ d1w') =============
                h1_ps = psum.tile([NUM_ROIS, J_LOC], F32, tag="misc", bufs=2)
                ki = 0
                for grp in range(n_d1_groups):
                    g = min(D1_G, n_k - grp * D1_G)
                    d1_t = d1wp.tile([128, D1_G * J_LOC], BIG, tag="d1w")
                    nc.sync.dma_start(
                        d1_t[:, :g * J_LOC],
                        d1w_d[:, grp * D1_G * J_LOC:
                              (grp * D1_G + g) * J_LOC])
                    for gi in range(g):
                        # k-chunk index ki = p_lin*OB + ob
                        p_lin, ob = divmod(ki, OB)
                        nc.tensor.matmul(
                            h1_ps[:],
                            redf[:, ob * NCOLS + p_lin:
                                 ob * NCOLS + NCOLS:NBINS],
                            d1_t[:, gi * J_LOC:(gi + 1) * J_LOC],
                            start=(ki == 0), stop=(ki == n_k - 1))
                        ki += 1
                if with_d1b:
                    nc.vector.tensor_add(h1_ps[:], h1_ps[:], d1b_sb[:])
                h1_sb = work.tile([NUM_ROIS, J_LOC], SML, tag="h1sb")
                nc.scalar.activation(h1_sb[:], h1_ps[:],
                                     mybir.ActivationFunctionType.Relu)

                # transpose h1 [4, 512] -> h1T [512, 4] via PE
                h1t_sb = work.tile([128, JB * NUM_ROIS], SML, tag="h1t")
                for jb in range(JB):
                    t_ps = psum.tile([128, NUM_ROIS], SML, tag="misc", bufs=2)
                    nc.tensor.transpose(
                        t_ps[:], h1_sb[:, jb * 128:(jb + 1) * 128],
                        ident_sb[:NUM_ROIS, :NUM_ROIS])
                    nc.vector.tensor_copy(
                        h1t_sb[:, jb * NUM_ROIS:(jb + 1) * NUM_ROIS], t_ps[:])

                # ================= d2: partial h2T [2048, 4] ===============
                h2t_ps = psum.tile([128, MB * NUM_ROIS], F32, tag="misc", bufs=2)
                for mb in range(MB):
                    for jb in range(JB):
                        nc.tensor.matmul(
                            h2t_ps[:, mb * NUM_ROIS:(mb + 1) * NUM_ROIS],
                            d2w_sb[jb][:, mb * 128:(mb + 1) * 128],
                            h1t_sb[:, jb * NUM_ROIS:(jb + 1) * NUM_ROIS],
                            start=(jb == 0), stop=(jb == JB - 1))
                h2t_part = work.tile([128, MB * NUM_ROIS], F32, tag="h2tp")
                nc.vector.tensor_copy(h2t_part[:], h2t_ps[:])

                # ================= AllReduce #2: h2T partials ==============
                nc.sync.dma_start(ar2_in[it][:], h2t_part[:])
                if no_collectives:
                    nc.sync.dma_start(ar2_out[it][:], ar2_in[it][:])
                else:
                    nc.gpsimd.collective_compute(
                        "AllReduce", mybir.AluOpType.add,
                        ins=[ar2_in[it][:]], outs=[ar2_out[it][:]],
                        replica_groups=[list(range(N_CORES))])
                h2t_sum = work.tile([128, MB * NUM_ROIS], F32, tag="h2ts")
                nc.sync.dma_start(h2t_sum[:], ar2_out[it][:])
                if with_d2b:
                    nc.vector.tensor_add(h2t_sum[:], h2t_sum[:], d2b_sb[:])
                h2r = work.tile([128, MB * NUM_ROIS], SML, tag="h2r")
                nc.scalar.activation(h2r[:], h2t_sum[:],
                                     mybir.ActivationFunctionType.Relu)

                # ================= d3/d4 head ==============================
                log_ps = psum.tile([NUM_ROIS, NOUT], F32, tag="misc", bufs=2)
                for kb in range(MB):
                    nc.tensor.matmul(
                        log_ps[:],
                        h2r[:, kb * NUM_ROIS:(kb + 1) * NUM_ROIS],
                        d34w_sb[:, kb * NOUT:(kb + 1) * NOUT],
                        start=(kb == 0), stop=(kb == MB - 1))

                # softmax over the first 21 columns
                mx = work.tile([NUM_ROIS, 1], F32, tag="mx")
                nc.vector.reduce_max(mx[:], log_ps[:, :NCLS],
                                     axis=mybir.AxisListType.X)
                negmx = work.tile([NUM_ROIS, 1], F32, tag="negmx")
                nc.vector.tensor_scalar_mul(negmx[:], mx[:], -1.0)
                expv = work.tile([NUM_ROIS, NCLS], F32, tag="expv")
                sume = work.tile([NUM_ROIS, 1], F32, tag="sume")
                nc.scalar.activation(expv[:], log_ps[:, :NCLS],
                                     mybir.ActivationFunctionType.Exp,
                                     bias=negmx[:], accum_out=sume[:])
                rinv = work.tile([NUM_ROIS, 1], F32, tag="rinv")
                nc.vector.reciprocal(rinv[:], sume[:])
                out_sb = work.tile([NUM_ROIS, NOUT], F32, tag="outsb")
                nc.scalar.activation(out_sb[:, :NCLS], expv[:],
                                     mybir.ActivationFunctionType.Copy,
                                     scale=rinv[:])
                nc.vector.tensor_copy(out_sb[:, NCLS:], log_ps[:, NCLS:])
                nc.sync.dma_start(out_d[:], out_sb[:])

    nc.compile()
    return nc


def _swz(a, inner):
    """[n*128, inner] row-major -> [128, n*inner] partition-major."""
    n = a.shape[0] // 128
    return np.ascontiguousarray(
        a.reshape(n, 128, inner).transpose(1, 0, 2).reshape(128, n * inner))


def _prepare_inputs(base_x, rois, conv_w, conv_b, d1_w, d1_b, d2_w, d2_b,
                    d3_w, d4_w, big_np, small_np):
    """Host-side sharding/layout prep. Returns (pxp_list, in_maps, flags)."""
    metas = _pool_meta(np.asarray(rois))
    pxp_list = [m["pxp"] for m in metas]

    base_x = np.asarray(base_x, np.float32)
    conv_w = np.asarray(conv_w, np.float32)
    d1_w = np.asarray(d1_w, np.float32)
    d2_w = np.asarray(d2_w, np.float32)
    d3_w = np.asarray(d3_w, np.float32)
    d4_w = np.asarray(d4_w, np.float32)
    conv_b = np.asarray(conv_b, np.float32)
    d1_b = np.asarray(d1_b, np.float32)
    d2_b = np.asarray(d2_b, np.float32)

    with_cb = bool(np.any(conv_b))
    with_d1b = bool(np.any(d1_b))
    with_d2b = bool(np.any(d2_b))

    inv_area = np.zeros((128, NCOLS), np.float32)
    for r, m in enumerate(metas):
        inv_area[:, r * NBINS:(r + 1) * NBINS] = m["inv_area"][None, :]
    ident = np.eye(128, dtype=small_np)

    # d1 column permutation: k' = (p_lin*OB + ob)*128 + o_in <- k = o*49 + p_lin
    o = np.arange(RED)
    p = np.arange(NBINS)
    ob, o_in = np.divmod(o, 128)
    # kprime row index for (p_lin, o): (p_lin*OB + ob)*128 + o_in
    kp = (p[:, None] * OB + ob[None, :]) * 128 + o_in[None, :]   # [49, 512]
    k_src = (o[None, :] * NBINS + p[:, None]).reshape(-1)         # [49*512]
    kp_flat = kp.reshape(-1)
    perm = np.empty(FLAT, np.int64)
    perm[kp_flat] = k_src                                          # d1wT row kp <- d1 col k_src

    d34 = np.concatenate([d3_w, d4_w], axis=0).T.astype(small_np)  # [2048, 101]
    d34 = _swz(np.ascontiguousarray(d34), NOUT)

    in_maps = []
    for c in range(N_CORES):
        csl = slice(c * C_LOC, (c + 1) * C_LOC)
        jsl = slice(c * J_LOC, (c + 1) * J_LOC)
        m = {}
        for r, meta in enumerate(metas):
            x, y, h, w = meta["x"], meta["y"], meta["h"], meta["w"]
            crop = base_x[0, csl, x:x + w, y:y + h].reshape(C_LOC, -1).T
            cpad = np.zeros((meta["pxp"], C_LOC), big_np)
            cpad[:meta["px"]] = crop.astype(big_np)
            m[f"crop{r}"] = _swz(cpad, C_LOC)
            m[f"b{r}"] = _swz(meta["B"].astype(big_np), NBINS)
        m["inv_area"] = inv_area
        m["ident"] = ident
        m["convw"] = np.ascontiguousarray(conv_w[:, csl].T.astype(np.float32))
        d1c = d1_w[jsl]                                            # [512, 25088]
        m["d1w"] = _swz(np.ascontiguousarray(d1c[:, perm].T.astype(big_np)), J_LOC)
        m["d2w"] = _swz(np.ascontiguousarray(d2_w[:, jsl].T.astype(small_np)), FEAT)
        m["d34w"] = d34
        if with_cb:
            cb_bc = np.zeros((128, OB * NCOLS), np.float32)
            for obi in range(OB):
                cb_bc[:, obi * NCOLS:(obi + 1) * NCOLS] = \
                    conv_b[obi * 128:(obi + 1) * 128][:, None]
            m["cbias"] = cb_bc
        if with_d1b:
            m["d1bias"] = np.broadcast_to(d1_b[jsl], (NUM_ROIS, J_LOC)).copy()
        if with_d2b:
            d2b_bc = np.zeros((128, MB * NUM_ROIS), np.float32)
            for mbi in range(MB):
                d2b_bc[:, mbi * NUM_ROIS:(mbi + 1) * NUM_ROIS] = \
                    d2_b[mbi * 128:(mbi + 1) * 128][:, None]
            m["d2bias"] = d2b_bc
        in_maps.append(m)
    return pxp_list, in_maps, (with_cb, with_d1b, with_d2b)


BIG_NP = ml_dtypes.bfloat16   # crops / B / d1w / redf
SMALL_NP = ml_dtypes.bfloat16  # d2w / d34w / h1 / h2 path (conv path pinned f32)


def get_program(inputs, big_np=None, small_np=None, reps=1,
                no_collectives=False):
    """Build (or fetch cached) program + prepared inputs for these rois."""
    big_np = big_np or BIG_NP
    small_np = small_np or SMALL_NP
    pxp_list, in_maps, flags = _prepare_inputs(big_np=big_np,
                                               small_np=small_np, **inputs)
    key = (tuple(pxp_list), np.dtype(big_np).str, np.dtype(small_np).str,
           reps, flags, no_collectives)
    if key not in _CACHE:
        _CACHE[key] = _build(pxp_list, big_np, small_np, reps=reps,
                             with_cb=flags[0], with_d1b=flags[1],
                             with_d2b=flags[2], no_collectives=no_collectives)
    return _CACHE[key], in_maps


def kernel(**inputs):
    inputs.pop("rois_unused", None)
    nc, in_maps = get_program(inputs)
    res = run_bass_kernel_spmd(nc, in_maps, core_ids=list(range(N_CORES)))
    out = np.asarray(res.results[0]["out"], np.float32)
    return out[:, :NCLS].copy(), out[:, NCLS:].copy()


# revision 11
# speedup vs baseline: 1.2874x; 1.2874x over previous
"""Trainium2 Bass kernel for nn_Classifier_55783035241052 (ROI pooling classifier).

Pipeline (reference):
  4 ROI crops of base_x[1,2048,128,128] -> adaptive_avg_pool 7x7 -> 1x1 conv
  2048->512 -> flatten [4,25088] -> d1 [4096,25088] relu -> d2 [2048,4096] relu
  -> d3 [21,2048] softmax / d4 [80,2048].

Sharding over 8 NeuronCores:
  - pooling/conv: channel-parallel (256 ch/core), pooling done as matmul with a
    host-built 0/1 binning matrix; conv contracts the local channels ->
    partial red [512,196]; AllReduce #1 sums partials.
  - d1: row-parallel (512 of 4096 outputs per core), weights streamed from HBM
    in a k'-permuted layout matching the on-chip red layout.
  - d2: contraction-parallel (512-col slice per core) producing partial
    h2^T [2048,4]; AllReduce #2 sums partials.
  - d3/d4 + softmax: replicated (tiny); host takes core 0's output.

kernel(**inputs) -> (out_class [4,21] f32, out_regr [4,80] f32)
"""
import numpy as np
import ml_dtypes

from concourse import bacc, mybir, tile
from concourse.bass_utils import run_bass_kernel_spmd

N_CORES = 8
POOL = 7
NBINS = POOL * POOL          # 49
NUM_ROIS = 4
FEAT = 2048
RED = 512
FLAT = RED * NBINS           # 25088
H1 = 2 * FEAT                # 4096
NCLS = 21
NREG = 4 * (NCLS - 1)        # 80
NOUT = NCLS + NREG           # 101

C_LOC = FEAT // N_CORES      # 256 channels per core
CB = C_LOC // 128            # 2 channel blocks per core
OB = RED // 128              # 4 conv-output blocks
J_LOC = H1 // N_CORES        # 512 d1 rows per core
JB = J_LOC // 128            # 4 j blocks
MB = FEAT // 128             # 16 h2 blocks
NCOLS = NUM_ROIS * NBINS     # 196 (r*49 + p) columns

F32 = mybir.dt.float32

_CACHE = {}


def _roi_b_matrix(S1h, S2h, du, dv, S1, S2):
    """0/1 matrix [S1h*S2h, 49] mapping HOST-crop pixels to this ROI's bins.
    The ROI occupies host rows [du, du+S1) and cols [dv, dv+S2)."""
    i = np.arange(POOL)
    hs = np.floor(i * S1 / POOL).astype(np.int64)
    he = np.ceil((i + 1) * S1 / POOL).astype(np.int64)
    ws = np.floor(i * S2 / POOL).astype(np.int64)
    we = np.ceil((i + 1) * S2 / POOL).astype(np.int64)
    u = np.arange(S1h) - du            # ROI-local row of each host row
    v = np.arange(S2h) - dv
    row_in = (u[:, None] >= hs[None, :]) & (u[:, None] < he[None, :])
    col_in = (v[:, None] >= ws[None, :]) & (v[:, None] < we[None, :])
    Bfull = (row_in[:, None, :, None] & col_in[None, :, None, :])
    area = ((he - hs)[:, None] * (we - ws)[None, :]).astype(np.float64)
    return (Bfull.reshape(S1h * S2h, NBINS).astype(np.float32),
            (1.0 / area).reshape(NBINS).astype(np.float32))


def _pool_meta(rois):
    """ROI geometry with containment grouping. rois rows are (x, y, h, w);
    crop = base_x[:, x:x+w, y:y+h], spatial dims (S1, S2) = (w, h).
    If ROI r's rectangle is contained in ROI s's, r is pooled against s's
    crop (no separate crop shipped)."""
    rects = []
    for rid in range(NUM_ROIS):
        x, y, h, w = (int(v) for v in rois[rid])
        rects.append((x, y, w, h))     # rows [x, x+w), cols [y, y+h)
    host = list(range(NUM_ROIS))
    for r in range(NUM_ROIS):
        xr, yr, wr, hr = rects[r]
        for t in range(NUM_ROIS):
            if t == r:
                continue
            xt, yt, wt, ht = rects[t]
            inside = (xt <= xr and xr + wr <= xt + wt and
                      yt <= yr and yr + hr <= yt + ht)
            # host must itself be a root (avoid chains/cycles deterministically)
            if inside and host[t] == t and (rects[t][2] * rects[t][3] >
                                            rects[r][2] * rects[r][3]):
                host[r] = t
                break
    groups = []                        # one entry per root roi
    for root in range(NUM_ROIS):
        if host[root] != root:
            continue
        members = [r for r in range(NUM_ROIS) if host[r] == root]
        x0, y0, w0, h0 = rects[root]
        px = w0 * h0
        pxp = ((px + 127) // 128) * 128
        Bcat = np.zeros((pxp, NBINS * len(members)), np.float32)
        invs = []
        for k, r in enumerate(members):
            xr, yr, wr, hr = rects[r]
            B, inv = _roi_b_matrix(w0, h0, xr - x0, yr - y0, wr, hr)
            Bcat[:px, k * NBINS:(k + 1) * NBINS] = B
            invs.append(inv)
        groups.append(dict(root=root, members=members, x=x0, y=y0,
                           w=w0, h=h0, px=px, pxp=pxp, B=Bcat, invs=invs))
    return groups


def _build(group_shapes, big_np, small_np, reps=1, with_cb=False,
           with_d1b=False, with_d2b=False, no_collectives=False):
    """Build the Bass program. group_shapes: per pooling group
    (pxp, n_members, member_rois). big_np: dtype for bandwidth-heavy
    tensors (crops, B, d1w, redf); small_np: the rest."""
    BIG = mybir.dt.from_np(np.dtype(big_np))
    SML = mybir.dt.from_np(np.dtype(small_np))
    nc = bacc.Bacc("TRN2", target_bir_lowering=False, debug=False,
                   num_devices=N_CORES)

    # ---- DRAM parameters (per-core data arrives via in_maps) ----
    n_groups = len(group_shapes)
    crop_d = [nc.dram_tensor(f"crop{g}", [128, (gs[0] // 128) * C_LOC],
                             BIG, kind="ExternalInput")
              for g, gs in enumerate(group_shapes)]
    b_d = [nc.dram_tensor(f"b{g}", [128, (gs[0] // 128) * NBINS * gs[1]], BIG,
                          kind="ExternalInput")
           for g, gs in enumerate(group_shapes)]
    invarea_d = nc.dram_tensor("inv_area", [128, NCOLS], F32, kind="ExternalInput")
    ident_d = nc.dram_tensor("ident", [128, 128], SML, kind="ExternalInput")
    convw_d = nc.dram_tensor("convw", [C_LOC, RED], F32, kind="ExternalInput")
    d1w_d = nc.dram_tensor("d1w", [128, (FLAT // 128) * J_LOC], BIG,
                           kind="ExternalInput")
    d2w_d = nc.dram_tensor("d2w", [128, JB * FEAT], SML, kind="ExternalInput")
    d34w_d = nc.dram_tensor("d34w", [128, MB * NOUT], SML, kind="ExternalInput")
    cb_d = nc.dram_tensor("cbias", [128, OB * NCOLS], F32,
                          kind="ExternalInput") if with_cb else None
    d1b_d = nc.dram_tensor("d1bias", [NUM_ROIS, J_LOC], F32,
                           kind="ExternalInput") if with_d1b else None
    d2b_d = nc.dram_tensor("d2bias", [128, MB * NUM_ROIS], F32,
                           kind="ExternalInput") if with_d2b else None
    out_d = nc.dram_tensor("out", [NUM_ROIS, NOUT], F32, kind="ExternalOutput")

    # Collective bounce buffers (internal DRAM; outputs must be Shared).
    ar1_in = [nc.dram_tensor(f"ar1_in{i}", [128, OB * NCOLS], F32)
              for i in range(reps)]
    ar1_out = [nc.dram_tensor(f"ar1_out{i}", [128, OB * NCOLS], F32,
                              addr_space="Shared") for i in range(reps)]
    ar2_in = [nc.dram_tensor(f"ar2_in{i}", [128, MB * NUM_ROIS], F32)
              for i in range(reps)]
    ar2_out = [nc.dram_tensor(f"ar2_out{i}", [128, MB * NUM_ROIS], F32,
                              addr_space="Shared") for i in range(reps)]

    is_bf16 = np.dtype(big_np).itemsize == 2
    CROP_G = 16   # px chunks per crop DMA
    D1_G = 8      # k-chunks per d1w DMA group (1 MB bf16)
    D1_BUFS = 8 if is_bf16 else 4
    CROP_BUFS = 3 if is_bf16 else 2
    n_k = FLAT // 128          # 196 contraction chunks for d1
    n_d1_groups = (n_k + D1_G - 1) // D1_G

    with tile.TileContext(nc) as tc:
        with (
            tc.tile_pool(name="const", bufs=1) as constp,
            tc.tile_pool(name="cropp", bufs=CROP_BUFS) as cropp,
            tc.tile_pool(name="bp", bufs=3) as bp,
            tc.tile_pool(name="d1wp", bufs=D1_BUFS) as d1wp,
            tc.tile_pool(name="work", bufs=2) as work,
            tc.tile_pool(name="psum", bufs=1, space="PSUM") as psum,
        ):
            # ---- resident constants ----
            invarea_sb = constp.tile([128, NCOLS], F32)
            nc.sync.dma_start(invarea_sb[:], invarea_d[:])
            ident_sb = constp.tile([128, 128], SML)
            nc.sync.dma_start(ident_sb[:], ident_d[:])
            convw_sb = [constp.tile([128, RED], F32, tag=f"convw{cb}", name=f"convw_sb{cb}")
                        for cb in range(CB)]
            for cb in range(CB):
                nc.sync.dma_start(convw_sb[cb][:],
                                  convw_d[cb * 128:(cb + 1) * 128, :])
            # (convw kept row-major: 2 KB contiguous rows)
            d2w_sb = [constp.tile([128, FEAT], SML, tag=f"d2w{jb}", name=f"d2w_sb{jb}")
                      for jb in range(JB)]
            for jb in range(JB):
                nc.sync.dma_start(d2w_sb[jb][:],
                                  d2w_d[:, jb * FEAT:(jb + 1) * FEAT])
            d34w_sb = constp.tile([128, MB * NOUT], SML)
            nc.sync.dma_start(d34w_sb[:], d34w_d[:])
            cb_sb = None
            if with_cb:
                cb_sb = constp.tile([128, OB * NCOLS], F32)
                nc.sync.dma_start(cb_sb[:], cb_d[:])
            d1b_sb = None
            if with_d1b:
                d1b_sb = constp.tile([NUM_ROIS, J_LOC], F32)
                nc.sync.dma_start(d1b_sb[:], d1b_d[:])
            d2b_sb = None
            if with_d2b:
                d2b_sb = constp.tile([128, MB * NUM_ROIS], F32)
                nc.sync.dma_start(d2b_sb[:], d2b_d[:])

            for it in range(reps):
                # ========== pooling (per group, per channel block) =========
                pooled_sb = [work.tile([128, NCOLS], F32, tag=f"pooled{cb}", name=f"pooled_sb{cb}")
                             for cb in range(CB)]
                for gidx, (pxp, nmem, members) in enumerate(group_shapes):
                    nchunks = pxp // 128
                    nb = NBINS * nmem
                    pool_ps = [psum.tile([128, NBINS * NUM_ROIS], F32,
                                         tag="accum", bufs=3,
                                         name=f"pool_ps{cb}")
                               for cb in range(CB)]
                    done = 0
                    while done < nchunks:
                        g = min(CROP_G, nchunks - done)
                        crop_t = cropp.tile([128, CROP_G * C_LOC], BIG, tag="crop")
                        nc.sync.dma_start(
                            crop_t[:, :g * C_LOC],
                            crop_d[gidx][:, done * C_LOC:(done + g) * C_LOC])
                        b_t = bp.tile([128, CROP_G * NBINS * NUM_ROIS], BIG,
                                      tag="bmat")
                        nc.sync.dma_start(
                            b_t[:, :g * nb],
                            b_d[gidx][:, done * nb:(done + g) * nb])
                        for gi in range(g):
                            ci = done + gi
                            for cb in range(CB):
                                nc.tensor.matmul(
                                    pool_ps[cb][:, :nb],
                                    crop_t[:, gi * C_LOC + cb * 128:
                                           gi * C_LOC + (cb + 1) * 128],
                                    b_t[:, gi * nb:(gi + 1) * nb],
                                    start=(ci == 0), stop=(ci == nchunks - 1))
                        done += g
                    # scale by 1/area, write each member's 49 columns
                    for k, r in enumerate(members):
                        for cb in range(CB):
                            nc.vector.tensor_tensor(
                                pooled_sb[cb][:, r * NBINS:(r + 1) * NBINS],
                                pool_ps[cb][:, k * NBINS:(k + 1) * NBINS],
                                invarea_sb[:, r * NBINS:(r + 1) * NBINS],
                                mybir.AluOpType.mult)

                # ================= 1x1 conv (local channels) ===============
                red_cat = work.tile([128, OB * NCOLS], F32, tag="red_cat")
                for ob in range(OB):
                    red_ps = psum.tile([128, NCOLS], F32, tag="accum", bufs=3)
                    for cb in range(CB):
                        nc.tensor.matmul(
                            red_ps[:],
                            convw_sb[cb][:, ob * 128:(ob + 1) * 128],
                            pooled_sb[cb][:],
                            start=(cb == 0), stop=(cb == CB - 1))
                    nc.vector.tensor_copy(
                        red_cat[:, ob * NCOLS:(ob + 1) * NCOLS], red_ps[:])

                # ================= AllReduce #1: red partials ==============
                nc.sync.dma_start(ar1_in[it][:], red_cat[:])
                if no_collectives:
                    nc.sync.dma_start(ar1_out[it][:], ar1_in[it][:])
                else:
                    nc.gpsimd.collective_compute(
                        "AllReduce", mybir.AluOpType.add,
                        ins=[ar1_in[it][:]], outs=[ar1_out[it][:]],
                        replica_groups=[list(range(N_CORES))])
                redsum = work.tile([128, OB * NCOLS], F32, tag="redsum")
                nc.sync.dma_start(redsum[:], ar1_out[it][:])
                if with_cb:
                    nc.vector.tensor_add(redsum[:], redsum[:], cb_sb[:])
                redf = work.tile([128, OB * NCOLS], BIG, tag="redf")
                nc.vector.tensor_copy(redf[:], redsum[:])

                # ================= d1: h1 = relu(flat' @ d1w') =============
                h1_ps = psum.tile([NUM_ROIS, J_LOC], F32, tag="misc", bufs=2)
                ki = 0
                for grp in range(n_d1_groups):
                    g = min(D1_G, n_k - grp * D1_G)
                    d1_t = d1wp.tile([128, D1_G * J_LOC], BIG, tag="d1w")
                    nc.sync.dma_start(
                        d1_t[:, :g * J_LOC],
                        d1w_d[:, grp * D1_G * J_LOC:
                              (grp * D1_G + g) * J_LOC])
                    for gi in range(g):
                        # k-chunk index ki = p_lin*OB + ob
                        p_lin, ob = divmod(ki, OB)
                        nc.tensor.matmul(
                            h1_ps[:],
                            redf[:, ob * NCOLS + p_lin:
                                 ob * NCOLS + NCOLS:NBINS],
                            d1_t[:, gi * J_LOC:(gi + 1) * J_LOC],
                            start=(ki == 0), stop=(ki == n_k - 1))
                        ki += 1
                if with_d1b:
                    nc.vector.tensor_add(h1_ps[:], h1_ps[:], d1b_sb[:])
                h1_sb = work.tile([NUM_ROIS, J_LOC], SML, tag="h1sb")
                nc.scalar.activation(h1_sb[:], h1_ps[:],
                                     mybir.ActivationFunctionType.Relu)

                # transpose h1 [4, 512] -> h1T [512, 4] via PE
                h1t_sb = work.tile([128, JB * NUM_ROIS], SML, tag="h1t")
                for jb in range(JB):
                    t_ps = psum.tile([128, NUM_ROIS], SML, tag="misc", bufs=2)
                    nc.tensor.transpose(
                        t_ps[:], h1_sb[:, jb * 128:(jb + 1) * 128],
                        ident_sb[:NUM_ROIS, :NUM_ROIS])
                    nc.vector.tensor_copy(
                        h1t_sb[:, jb * NUM_ROIS:(jb + 1) * NUM_ROIS], t_ps[:])

                # ================= d2: partial h2T [2048, 4] ===============
                h2t_ps = psum.tile([128, MB * NUM_ROIS], F32, tag="misc", bufs=2)
                for mb in range(MB):
                    for jb in range(JB):
                        nc.tensor.matmul(
                            h2t_ps[:, mb * NUM_ROIS:(mb + 1) * NUM_ROIS],
                            d2w_sb[jb][:, mb * 128:(mb + 1) * 128],
                            h1t_sb[:, jb * NUM_ROIS:(jb + 1) * NUM_ROIS],
                            start=(jb == 0), stop=(jb == JB - 1))
                h2t_part = work.tile([128, MB * NUM_ROIS], F32, tag="h2tp")
                nc.vector.tensor_copy(h2t_part[:], h2t_ps[:])

                # ================= AllReduce #2: h2T partials ==============
                nc.sync.dma_start(ar2_in[it][:], h2t_part[:])
                if no_collectives:
                    nc.sync.dma_start(ar2_out[it][:], ar2_in[it][:])
                else:
                    nc.gpsimd.collective_compute(
                        "AllReduce", mybir.AluOpType.add,
                        ins=[ar2_in[it][:]], outs=[ar2_out[it][:]],
                        replica_groups=[list(range(N_CORES))])
                h2t_sum = work.tile([128, MB * NUM_ROIS], F32, tag="h2ts")
                nc.sync.dma_start(h2t_sum[:], ar2_out[it][:])
                if with_d2b:
                    nc.vector.tensor_add(h2t_sum[:], h2t_sum[:], d2b_sb[:])
                h2r = work.tile([128, MB * NUM_ROIS], SML, tag="h2r")
                nc.scalar.activation(h2r[:], h2t_sum[:],
                                     mybir.ActivationFunctionType.Relu)

                # ================= d3/d4 head ==============================
                log_ps = psum.tile([NUM_ROIS, NOUT], F32, tag="misc", bufs=2)
                for kb in range(MB):
                    nc.tensor.matmul(
                        log_ps[:],
                        h2r[:, kb * NUM_ROIS:(kb + 1) * NUM_ROIS],
                        d34w_sb[:, kb * NOUT:(kb + 1) * NOUT],
                        start=(kb == 0), stop=(kb == MB - 1))

                # softmax over the first 21 columns
                mx = work.tile([NUM_ROIS, 1], F32, tag="mx")
                nc.vector.reduce_max(mx[:], log_ps[:, :NCLS],
                                     axis=mybir.AxisListType.X)
                negmx = work.tile([NUM_ROIS, 1], F32, tag="negmx")
                nc.vector.tensor_scalar_mul(negmx[:], mx[:], -1.0)
                expv = work.tile([NUM_ROIS, NCLS], F32, tag="expv")
                sume = work.tile([NUM_ROIS, 1], F32, tag="sume")
                nc.scalar.activation(expv[:], log_ps[:, :NCLS],
                                     mybir.ActivationFunctionType.Exp,
                                     bias=negmx[:], accum_out=sume[:])
                rinv = work.tile([NUM_ROIS, 1], F32, tag="rinv")
                nc.vector.reciprocal(rinv[:], sume[:])
                out_sb = work.tile([NUM_ROIS, NOUT], F32, tag="outsb")
                nc.scalar.activation(out_sb[:, :NCLS], expv[:],
                                     mybir.ActivationFunctionType.Copy,
                                     scale=rinv[:])
                nc.vector.tensor_copy(out_sb[:, NCLS:], log_ps[:, NCLS:])
                nc.sync.dma_start(out_d[:], out_sb[:])

    nc.compile()
    return nc


def _swz(a, inner):
    """[n*128, inner] row-major -> [128, n*inner] partition-major."""
    n = a.shape[0] // 128
    return np.ascontiguousarray(
        a.reshape(n, 128, inner).transpose(1, 0, 2).reshape(128, n * inner))


def _prepare_inputs(base_x, rois, conv_w, conv_b, d1_w, d1_b, d2_w, d2_b,
                    d3_w, d4_w, big_np, small_np):
    """Host-side sharding/layout prep. Returns (pxp_list, in_maps, flags)."""
    groups = _pool_meta(np.asarray(rois))
    group_shapes = tuple((g["pxp"], len(g["members"]), tuple(g["members"]))
                         for g in groups)

    base_x = np.asarray(base_x, np.float32)
    conv_w = np.asarray(conv_w, np.float32)
    d1_w = np.asarray(d1_w, np.float32)
    d2_w = np.asarray(d2_w, np.float32)
    d3_w = np.asarray(d3_w, np.float32)
    d4_w = np.asarray(d4_w, np.float32)
    conv_b = np.asarray(conv_b, np.float32)
    d1_b = np.asarray(d1_b, np.float32)
    d2_b = np.asarray(d2_b, np.float32)

    with_cb = bool(np.any(conv_b))
    with_d1b = bool(np.any(d1_b))
    with_d2b = bool(np.any(d2_b))

    inv_area = np.zeros((128, NCOLS), np.float32)
    for g in groups:
        for k, r in enumerate(g["members"]):
            inv_area[:, r * NBINS:(r + 1) * NBINS] = g["invs"][k][None, :]
    ident = np.eye(128, dtype=small_np)

    # d1 column permutation: k' = (p_lin*OB + ob)*128 + o_in <- k = o*49 + p_lin
    o = np.arange(RED)
    p = np.arange(NBINS)
    ob, o_in = np.divmod(o, 128)
    # kprime row index for (p_lin, o): (p_lin*OB + ob)*128 + o_in
    kp = (p[:, None] * OB + ob[None, :]) * 128 + o_in[None, :]   # [49, 512]
    k_src = (o[None, :] * NBINS + p[:, None]).reshape(-1)         # [49*512]
    kp_flat = kp.reshape(-1)
    perm = np.empty(FLAT, np.int64)
    perm[kp_flat] = k_src                                          # d1wT row kp <- d1 col k_src

    d34 = np.concatenate([d3_w, d4_w], axis=0).T.astype(small_np)  # [2048, 101]
    d34 = _swz(np.ascontiguousarray(d34), NOUT)

    in_maps = []
    for c in range(N_CORES):
        csl = slice(c * C_LOC, (c + 1) * C_LOC)
        jsl = slice(c * J_LOC, (c + 1) * J_LOC)
        m = {}
        for gidx, grp in enumerate(groups):
            x, y, w, h = grp["x"], grp["y"], grp["w"], grp["h"]
            crop = base_x[0, csl, x:x + w, y:y + h].reshape(C_LOC, -1).T
            cpad = np.zeros((grp["pxp"], C_LOC), big_np)
            cpad[:grp["px"]] = crop.astype(big_np)
            m[f"crop{gidx}"] = _swz(cpad, C_LOC)
            m[f"b{gidx}"] = _swz(grp["B"].astype(big_np),
                                 NBINS * len(grp["members"]))
        m["inv_area"] = inv_area
        m["ident"] = ident
        m["convw"] = np.ascontiguousarray(conv_w[:, csl].T.astype(np.float32))
        d1c = d1_w[jsl]                                            # [512, 25088]
        m["d1w"] = _swz(np.ascontiguousarray(d1c[:, perm].T.astype(big_np)), J_LOC)
        m["d2w"] = _swz(np.ascontiguousarray(d2_w[:, jsl].T.astype(small_np)), FEAT)
        m["d34w"] = d34
        if with_cb:
            cb_bc = np.zeros((128, OB * NCOLS), np.float32)
            for obi in range(OB):
                cb_bc[:, obi * NCOLS:(obi + 1) * NCOLS] = \
                    conv_b[obi * 128:(obi + 1) * 128][:, None]
            m["cbias"] = cb_bc
        if with_d1b:
            m["d1bias"] = np.broadcast_to(d1_b[jsl], (NUM_ROIS, J_LOC)).copy()
        if with_d2b:
            d2b_bc = np.zeros((128, MB * NUM_ROIS), np.float32)
            for mbi in range(MB):
                d2b_bc[:, mbi * NUM_ROIS:(mbi + 1) * NUM_ROIS] = \
                    d2_b[mbi * 128:(mbi + 1) * 128][:, None]
            m["d2bias"] = d2b_bc
        in_maps.append(m)
    return group_shapes, in_maps, (with_cb, with_d1b, with_d2b)


BIG_NP = ml_dtypes.bfloat16   # crops / B / d1w / redf
SMALL_NP = ml_dtypes.bfloat16  # d2w / d34w / h1 / h2 path (conv path pinned f32)


def get_program(inputs, big_np=None, small_np=None, reps=1,
                no_collectives=False):
    """Build (or fetch cached) program + prepared inputs for these rois."""
    big_np = big_np or BIG_NP
    small_np = small_np or SMALL_NP
    group_shapes, in_maps, flags = _prepare_inputs(big_np=big_np,
                                                   small_np=small_np, **inputs)
    key = (group_shapes, np.dtype(big_np).str, np.dtype(small_np).str,
           reps, flags, no_collectives)
    if key not in _CACHE:
        _CACHE[key] = _build(group_shapes, big_np, small_np, reps=reps,
                             with_cb=flags[0], with_d1b=flags[1],
                             with_d2b=flags[2], no_collectives=no_collectives)
    return _CACHE[key], in_maps


def kernel(**inputs):
    inputs.pop("rois_unused", None)
    nc, in_maps = get_program(inputs)
    res = run_bass_kernel_spmd(nc, in_maps, core_ids=list(range(N_CORES)))
    out = np.asarray(res.results[0]["out"], np.float32)
    return out[:, :NCLS].copy(), out[:, NCLS:].copy()
